# revision 4
# baseline (speedup 1.0000x reference)
"""AlphaCompositor Trainium2 kernel (8 NeuronCores, data-parallel over pixels).

v2: gather via InstDMAGatherAnt (SWDGE vectorized descriptor generation)
instead of per-column indirect_dma_start calls.

Reference computation:
    wts     = alphas * exclusive_cumprod(1-alphas, axis=K)
    feats   = ptclds.T[fragments]                  # random gather, (N,K,H,W,C)
    images  = einsum('nkhw,nkhwc->nchw', wts, feats)

Device strategy (per core, 32768 pixels, 16 chunks of 2048):
  - K=8 split as k = l*4 + k4; partition q = k4*32 + pb; slot j = (l,st,g).
  - Table stored as bf16 8-row blocks: blk[b, c*8+r] = ptclds[c, 8b+r]
    (512B elements, block index fits int16 since P/8 = 25000 < 32768).
  - Per chunk, 2 dma_gather calls (one per l) of 8192 indexed 512B elements
    each land F[q, j, (c, r)] in pixel-major slot order.
  - 1-of-8 row selection is folded into the weights: wsel[q, j, r] =
    w[q,j] * (frag&7 == r); T = reduce_r(F * wsel) on DVE.
  - Weights (exp of masked cumsum of logs via TensorE mask matmuls) are
    computed for all 16 chunks upfront in one batched pass.
  - Compositing sum over k: same selection matmuls as v1 (sel sums
    partitions with equal pb; l accumulated in PSUM).
"""
import sys

sys.path.insert(0, "/opt/trn_rl_repo")
import numpy as np
import ml_dtypes

N, K, H, W = 4, 8, 256, 256
C, P = 32, 200000
NCORES = 8
PIX_CORE = (N * H * W) // NCORES  # 32768
L = 2
K4 = 4
PB = 32
G = 16
CH_ST = 4
FD = L * CH_ST * G  # 128
N_CHUNK = 16
NBLK4 = P // 4  # 50000 4-row blocks of 256B
BIAS = 32768    # signed-int16 bias: stored idx = blk - BIAS, base += BIAS
# 8 single-packet 1024-idx calls per half; no pad slots: host permutes
# pixels so every call's last index position holds a fragment with
# block >= BIAS (stored >= 0), keeping the trailing-negative trim off
IDXCOLS = 1024

_STATE = {}


def _fix_compiler_flags():
    from concourse.compiler_utils import get_compiler_flags, set_compiler_flags

    fl = get_compiler_flags()
    out, mode = [], None
    for tok in fl:
        if tok.startswith("--"):
            mode = tok
        elif mode == "--internal-disable-dge-levels" and \
                tok == "vector_dynamic_offsets":
            continue
        out.append(tok)
    if "--internal-enable-dge-levels" in out:
        out.insert(out.index("--internal-enable-dge-levels") + 1,
                   "vector_dynamic_offsets")
    else:
        out += ["--internal-enable-dge-levels", "vector_dynamic_offsets"]
    set_compiler_flags(out)


def _build_graph():
    from concourse import bacc, bass, mybir
    import concourse.tile as tile

    _fix_compiler_flags()

    f32 = mybir.dt.float32
    bf16 = mybir.dt.bfloat16
    i16 = mybir.dt.int16

    nc = bacc.Bacc("TRN2", target_bir_lowering=False, debug=False,
                   num_devices=NCORES, num_swdge_queues=4)

    blkb_ext = nc.dram_tensor("blkb", [NBLK4, 128], bf16,
                              kind="ExternalInput").ap()
    idx_ext = nc.dram_tensor("idx", [N_CHUNK, 128, 1024], i16,
                             kind="ExternalInput").ap()
    # [q, (l, ch, st, g)] f32
    alpha_ext = nc.dram_tensor("alpha", [128, L, N_CHUNK, 64], f32,
                               kind="ExternalInput").ap()
    ridx_ext = nc.dram_tensor("ridx", [128, L, N_CHUNK, 64], f32,
                              kind="ExternalInput").ap()
    m1_ext = nc.dram_tensor("m1", [128, 128], f32, kind="ExternalInput").ap()
    mall_ext = nc.dram_tensor("mall", [128, 128], f32,
                              kind="ExternalInput").ap()
    idt_ext = nc.dram_tensor("idt", [128, 128], f32, kind="ExternalInput").ap()
    sel_ext = nc.dram_tensor("sel", [128, PB], bf16,
                             kind="ExternalInput").ap()
    iota8_ext = nc.dram_tensor("iota8", [128, 4], f32,
                               kind="ExternalInput").ap()
    out_ext = nc.dram_tensor("out", [N_CHUNK, 128, G * C], f32,
                             kind="ExternalOutput").ap()

    def bcast_mid(ap_obj, n_mid):
        """[128, a, b] AP -> [128, a, n_mid, b] with stride-0 middle axis."""
        (p0, p1), rest = ap_obj.ap[0], list(ap_obj.ap[1:])
        new = [(p0, p1)] + rest[:-1] + [(0, n_mid)] + [rest[-1]]
        return bass.AP(ap_obj.tensor, ap_obj.offset, new)

    with tile.TileContext(nc) as tc:
        with (
            tc.tile_pool(name="const", bufs=1) as cpool,
            tc.tile_pool(name="up", bufs=1) as upool,
            tc.tile_pool(name="idxp", bufs=3) as ipool,
            tc.tile_pool(name="wselp", bufs=3) as wpool,
            tc.tile_pool(name="feat", bufs=3) as fpool,
            tc.tile_pool(name="t8", bufs=1) as t8pool,
            tc.tile_pool(name="tp", bufs=3) as tpool,
            tc.tile_pool(name="evac", bufs=3) as opool,
            tc.tile_pool(name="psw", bufs=1, space="PSUM") as pswpool,
            tc.tile_pool(name="pso", bufs=2, space="PSUM") as psopool,
        ):
            m1_t = cpool.tile([128, 128], f32, tag="m1")
            nc.sync.dma_start(out=m1_t[:], in_=m1_ext[:])
            mall_t = cpool.tile([128, 128], f32, tag="mall")
            nc.sync.dma_start(out=mall_t[:], in_=mall_ext[:])
            idt_t = cpool.tile([128, 128], f32, tag="idt")
            nc.sync.dma_start(out=idt_t[:], in_=idt_ext[:])
            sel_t = cpool.tile([128, PB], bf16, tag="sel")
            nc.sync.dma_start(out=sel_t[:], in_=sel_ext[:])
            iota8_t = cpool.tile([128, 4], f32, tag="iota8")
            nc.sync.dma_start(out=iota8_t[:], in_=iota8_ext[:])
            eps_t = cpool.tile([128, 1], f32, tag="eps")
            nc.vector.memset(eps_t[:], 1e-37)

            # ---- upfront: weights for all chunks -------------------------
            a_all = upool.tile([128, L, N_CHUNK, 64], f32, tag="a_all")
            nc.sync.dma_start(out=a_all[:], in_=alpha_ext[:])
            ridx_all = upool.tile([128, L, N_CHUNK, 64], f32, tag="ridx_all")
            nc.sync.dma_start(out=ridx_all[:], in_=ridx_ext[:])

            ln_t = upool.tile([128, L, N_CHUNK, 64], f32, tag="ln_t")
            nc.scalar.activation(ln_t[:], a_all[:],
                                 mybir.ActivationFunctionType.Ln,
                                 bias=1.0, scale=-1.0)
            wt_all = upool.tile([128, L, N_CHUNK, 64], f32, tag="wt_all")

            def half(ap4, l, hh):  # [128, L, 16, 64] -> [128, 512]
                return ap4[:, l, hh * 8:(hh + 1) * 8, :].rearrange(
                    "p c m -> p (c m)")

            def ahalf(ap3, hh):  # [128, 16, 64] -> [128, 512]
                return ap3[:, hh * 8:(hh + 1) * 8, :].rearrange(
                    "p c m -> p (c m)")

            for l in range(L):
                ln_a = upool.tile([128, N_CHUNK, 64], f32, tag="ln_a")
                nc.scalar.activation(ln_a[:], a_all[:, l],
                                     mybir.ActivationFunctionType.Ln,
                                     bias=eps_t[:], scale=1.0)
                for hh in range(2):
                    psw = pswpool.tile([128, 512], f32, tag=f"psw{l}_{hh}")
                    nc.tensor.matmul(out=psw[:], lhsT=m1_t[:],
                                     rhs=half(ln_t, l, hh),
                                     start=True, stop=False)
                    if l == 1:
                        nc.tensor.matmul(out=psw[:], lhsT=mall_t[:],
                                         rhs=half(ln_t, 0, hh),
                                         start=False, stop=False)
                    nc.tensor.matmul(out=psw[:], lhsT=idt_t[:],
                                     rhs=ahalf(ln_a, hh),
                                     start=False, stop=True)
                    nc.scalar.activation(half(wt_all, l, hh), psw[:],
                                         mybir.ActivationFunctionType.Exp)

            # ---- per-chunk pipeline --------------------------------------
            for ch in range(N_CHUNK):
                idx = ipool.tile([128, 1024], i16, tag="idx")
                nc.sync.dma_start(out=idx[:], in_=idx_ext[ch])

                # wsel[q, l, (st,g), r] = w * (ridx == r)
                rview = ridx_all[:, :, ch, :]   # [128, 2, 64]
                wview = wt_all[:, :, ch, :]
                iota_b = bass.AP(
                    iota8_t[:].tensor, iota8_t[:].offset,
                    [iota8_t[:].ap[0], (0, 2), (0, 64), (1, 4)])
                mask8 = wpool.tile([128, L, 64, 4], bf16, tag="mask8")
                nc.vector.tensor_tensor(
                    out=mask8[:],
                    in0=rview.to_broadcast([128, 2, 64, 4]),
                    in1=iota_b,
                    op=mybir.AluOpType.is_equal)
                wsel = wpool.tile([128, L, 64, 4], bf16, tag="wsel")
                nc.vector.tensor_tensor(
                    out=wsel[:], in0=mask8[:],
                    in1=wview.to_broadcast([128, 2, 64, 4]),
                    op=mybir.AluOpType.mult)

                T = tpool.tile([128, L, CH_ST, G, C], bf16, tag="T")
                for h in range(2):
                    F = fpool.tile([128, 64, 128], bf16, tag="F")
                    for s in range(8):
                        nc.gpsimd.dma_gather(
                            F[:, s * 8:(s + 1) * 8, :], blkb_ext[BIAS:, :],
                            idx[:, h * 512 + s * 64:h * 512 + (s + 1) * 64],
                            num_idxs=1024, num_idxs_reg=1024,
                            elem_size=128,
                            queue_num=(h * 8 + s) % 4,
                        )
                    T8 = t8pool.tile([128, 64, 32, 4], bf16, tag="T8")
                    # sub-halves: product of slots 0-31 starts after the
                    # first 4 gather calls instead of all 8
                    for g2 in range(2):
                        sl = slice(g2 * 32, (g2 + 1) * 32)
                        Fv = F[:, sl, :].rearrange(
                            "p j (c r) -> p j c r", c=32, r=4)
                        wv = bcast_mid(wsel[:, h, sl, :], 32)
                        nc.vector.tensor_tensor(
                            out=T8[:, sl], in0=Fv, in1=wv,
                            op=mybir.AluOpType.mult)
                        with nc.allow_low_precision("bf16 T; 4-term r-sum"):
                            nc.vector.tensor_reduce(
                                out=T[:, h, 2 * g2:2 * g2 + 2],
                                in_=T8[:, sl],
                                axis=mybir.AxisListType.X,
                                op=mybir.AluOpType.add)

                pso = psopool.tile([128, G * C], f32, tag="pso")
                for st in range(CH_ST):
                    nc.tensor.matmul(
                        out=pso[st * PB:(st + 1) * PB, :], lhsT=sel_t[:],
                        rhs=T[:, 0, st].rearrange("p g c -> p (g c)"),
                        start=True, stop=False, tile_position=(0, st * PB))
                    nc.tensor.matmul(
                        out=pso[st * PB:(st + 1) * PB, :], lhsT=sel_t[:],
                        rhs=T[:, 1, st].rearrange("p g c -> p (g c)"),
                        start=False, stop=True, tile_position=(0, st * PB))

                evac = opool.tile([128, G * C], f32, tag="evac")
                nc.scalar.copy(evac[:], pso[:])
                nc.sync.dma_start(out=out_ext[ch], in_=evac[:])

    nc.compile()
    return nc


def _host_masks():
    kk = np.arange(128) // PB
    bb = np.arange(128) % PB
    m1 = ((bb[:, None] == bb[None, :]) & (kk[:, None] < kk[None, :]))
    mall = (bb[:, None] == bb[None, :])
    idt = np.eye(128)
    sel = (bb[:, None] == np.arange(PB)[None, :])
    iota8 = np.broadcast_to(np.arange(4, dtype=np.float32), (128, 4))
    return (m1.astype(np.float32), mall.astype(np.float32),
            idt.astype(np.float32), sel.astype(np.float32),
            np.ascontiguousarray(iota8))


def _host_layout(arr_k_pix):
    """[K, 32768] -> [N_CHUNK, 128=(k4,pb), FD=(l,st,g)] for one core."""
    x = arr_k_pix.reshape(L, K4, N_CHUNK, CH_ST, PB, G)
    x = x.transpose(2, 1, 4, 0, 3, 5)  # ch, k4, pb, l, st, g
    return np.ascontiguousarray(x.reshape(N_CHUNK, 128, FD))


def _wrap_idx_halves(fr_layout):
    """[N_CHUNK, 128, FD] int32 -> [N_CHUNK, 128, 2*NIC16] i16 idx tiles.

    Gather list for (ch, l): position i = (st*16+g)*128 + q, value
    (frag >> 2) - BIAS (signed trick), padded with 128 positive dummies
    (stored 0 = block BIAS) so the trailing-negative trim never fires;
    wrapped at [i%16, i//16] and replicated to all 8 16-partition groups.
    """
    blk = ((fr_layout >> 2) - BIAS).astype(np.int16)
    x = blk.reshape(N_CHUNK, 128, L, 64)  # ch, q, l, (st g)
    out = np.empty((N_CHUNK, 128, 1024), dtype=np.int16)
    for ch in range(N_CHUNK):
        for l in range(L):
            lin = x[ch, :, l, :].T.reshape(8192)  # [(st g), q]
            w16 = lin.reshape(512, 16).T
            out[ch, :, l * 512:(l + 1) * 512] = np.tile(w16, (8, 1))
    return out


def _qlcsg(arr_layout):
    """[N_CHUNK, 128, FD] -> [128, L, N_CHUNK, 64] contiguous."""
    x = arr_layout.reshape(N_CHUNK, 128, L, 64)
    return np.ascontiguousarray(x.transpose(1, 2, 0, 3))


def _fix_call_ends(fr, al):
    """Permute pixels within each 2048-pixel chunk so every gather call's
    last index position (partition 127 = (k4=3, pb=31), slot 8s+7) holds a
    fragment with block >= BIAS for both k=3 (l=0) and k=7 (l=1). Returns
    (fr', al', perms) with perms[ch] mapping position -> source pixel."""
    BIG = BIAS * 4
    endpos = [(st * 32 + 31) * 16 + g for st in range(4) for g in (7, 15)]
    perms = np.tile(np.arange(2048), (N_CHUNK, 1))
    for ch in range(N_CHUNK):
        base = ch * 2048
        f = fr[:, base:base + 2048]
        ok = (f[3] >= BIG) & (f[7] >= BIG)
        perm = perms[ch]
        cand = [q for q in np.flatnonzero(ok) if q not in endpos]
        ci = 0
        for p in endpos:
            if ok[perm[p]]:
                continue
            perm[p], perm[cand[ci]] = perm[cand[ci]], perm[p]
            ci += 1
        src = base + perm
        fr[:, base:base + 2048] = fr[:, src]
        al[:, base:base + 2048] = al[:, src]
    return fr, al, perms


def kernel(fragments, alphas, ptclds):
    if "nc" not in _STATE:
        _STATE["nc"] = _build_graph()
    nc = _STATE["nc"]
    from concourse.bass_utils import run_bass_kernel_spmd

    fragments = np.asarray(fragments)
    alphas = np.asarray(alphas, dtype=np.float32)
    ptclds = np.asarray(ptclds, dtype=np.float32)

    valid = fragments >= 0
    frag_i32 = np.where(valid, fragments, 0).astype(np.int32)
    alpha_f = np.where(valid, alphas, 0.0).astype(np.float32)

    # blocked bf16 table: blk[b, c*4+r] = ptclds[c, 4b+r]
    blkb = np.ascontiguousarray(
        ptclds.reshape(C, NBLK4, 4).transpose(1, 0, 2).reshape(NBLK4, 128)
    ).astype(ml_dtypes.bfloat16)

    m1, mall, idt, sel, iota8 = _host_masks()

    in_maps = []
    core_perms = []
    for core in range(NCORES):
        n0, hh = core // 2, core % 2
        fr = frag_i32[n0, :, hh * 128:(hh + 1) * 128, :].reshape(
            K, PIX_CORE).copy()
        al = alpha_f[n0, :, hh * 128:(hh + 1) * 128, :].reshape(
            K, PIX_CORE).copy()
        fr, al, perms = _fix_call_ends(fr, al)
        core_perms.append(perms)
        fr_l = _host_layout(fr)
        al_l = _host_layout(al)
        in_maps.append({
            "blkb": blkb,
            "idx": _wrap_idx_halves(fr_l),
            "alpha": _qlcsg(al_l),
            "ridx": _qlcsg((fr_l & 3).astype(np.float32)),
            "m1": m1, "mall": mall, "idt": idt,
            "sel": sel.astype(ml_dtypes.bfloat16), "iota8": iota8,
        })

    _STATE["last_in_maps"] = in_maps
    res = run_bass_kernel_spmd(nc, in_maps, list(range(NCORES)))

    per_core = []
    for i in range(NCORES):
        r = res.results[i]["out"].reshape(PIX_CORE, C)
        unperm = np.empty_like(r)
        for ch in range(N_CHUNK):
            base = ch * 2048
            unperm[base + core_perms[i][ch]] = r[base:base + 2048]
        per_core.append(unperm)
    full = np.stack(per_core).reshape(N, 2, 128, W, C)
    images = full.transpose(0, 4, 1, 2, 3).reshape(N, C, H, W)
    return np.ascontiguousarray(images)
